# revision 18
# baseline (speedup 1.0000x reference)
"""Trainium2 Bass kernel for a PointNet++-style feature-propagation decoder
(4 stages of kNN(k=3) inverse-distance-weighted feature interpolation).

Sharding: batch b = core//2 (data parallel over B=4); the finest stage's 8192
query points split in half across each core pair (point parallel along N per
the sharding hint). Stages 0-2 are duplicated within a pair; stage 3 is
n-split. Output rows 0:64 are the x0 passthrough, assembled on the host.

Per-core device pipeline per stage (all feature traffic in fp16):
  - negated partial distances nd = 2a.b - |b|^2 via one K=4 fp32 PE matmul
    per 128-query tile (the -|a|^2 term is constant per query and cannot
    change each query's top-k order; it is restored only for the 3 selected
    values when computing weights). |b|^2 rows come from a tiny ones-matmul.
  - top-3 neighbors via DVE max (top-8) + max_index read directly off PSUM
  - inverse-distance weights on DVE (batched per 8-tile block)
  - feature gather via SWDGE indirect DMA from fp16 DRAM tables
  - weighted 3-way combine on the PE: psum += diag(w_k) @ gathered_k, where
    diag(w_k) = identity * w_k built by one 4x-mode DVE tensor_scalar each
  - stage results are written as fp16 table rows for the next stage's gather;
    the final stage writes fp16 [n, d] rows that the host transposes.
"""

import numpy as np

P = 128
KNN = 3
EPS = 1e-8
DFLOOR = 1e-7
LAG = 3

B = 4
NS = [8192, 2048, 512, 128, 32]  # points per level, finest -> coarsest
CS = [64, 128, 256, 512, 1024]   # feature channels per level

_CACHED = {"nc": None, "key": None}


def _build_program(ns, cs, n_half, split_waits=True):
    """Trace the per-core Bass program. ns/cs as in reference (finest first).
    n_half: number of finest-level query points this core handles."""
    import contextlib

    import concourse.bass as bass
    import concourse.mybir as mybir
    import concourse.tile as tile
    from concourse.bass import IndirectOffsetOnAxis
    from concourse.masks import make_identity

    _patch_tile_drain()

    f32 = mybir.dt.float32
    f16 = mybir.dt.float16
    bf16 = mybir.dt.bfloat16
    u32 = mybir.dt.uint32
    Alu = mybir.AluOpType
    Axis = mybir.AxisListType

    # stage s: fine level (3-s) [this core's half of level 0 for s=3],
    # coarse level (4-s). d2[s]: width of the table gathered at stage s.
    d2 = [cs[4]]
    for s in range(1, 4):
        d2.append(cs[4 - s] + d2[s - 1])
    stages = []
    for s in range(4):
        nf = ns[3 - s] if s < 3 else n_half
        stages.append(dict(nf=nf, S=ns[4 - s], d2=d2[s],
                           cx=(cs[3 - s] if s < 3 else None)))

    nc = bass.Bass("TRN2")

    # ---- external inputs (per core) ----
    xt = {}   # xyz transposed [3, N] f32
    xr = {}   # xyz rows [N, 3] f32
    asp, bsp = {}, {}
    for i, n in enumerate(ns):
        nn_ = n_half if i == 0 else n
        xt[i] = nc.dram_tensor(f"xt{i}", [3, nn_], f32, kind="ExternalInput")
        xr[i] = nc.dram_tensor(f"xr{i}", [nn_, 3], f32, kind="ExternalInput")
        asp[i] = [nc.dram_tensor(f"a{j}_{i}", [3, nn_], bf16,
                                 kind="ExternalInput") for j in range(3)]
        if i > 0:
            bsp[i] = [nc.dram_tensor(f"b{j}_{i}", [3, n], bf16,
                                     kind="ExternalInput") for j in range(3)]
    ft = {}   # features transposed [N, C] fp16
    for i in range(1, 5):
        ft[i] = nc.dram_tensor(f"ft{i}", [ns[i], cs[i]], f16,
                               kind="ExternalInput")

    onesd = nc.dram_tensor("ones", [3, max(n_half, ns[1])], bf16,
                           kind="ExternalInput")

    # ---- external output: interp part of final stage, fp16 rows ----
    oi = nc.dram_tensor("oi", [n_half, d2[3]], f16, kind="ExternalOutput")

    # ---- internal fp16 DRAM source tables (stage 0 gathers ft[4]) ----
    tsrc = {s: nc.dram_tensor(f"tsrc{s}", [stages[s]["S"], stages[s]["d2"]],
                              f16)
            for s in range(1, 4)}
    gsrc = {0: ft[4], 1: tsrc[1], 2: tsrc[2], 3: tsrc[3]}

    with tile.TileContext(nc) as tc, contextlib.ExitStack() as ctx:
        cpool = ctx.enter_context(tc.tile_pool(name="const", bufs=1))
        abpool = ctx.enter_context(tc.tile_pool(name="ab", bufs=2))
        smpool = ctx.enter_context(tc.tile_pool(name="sm", bufs=2))
        gpool = ctx.enter_context(tc.tile_pool(name="g", bufs=5))
        dpool = ctx.enter_context(tc.tile_pool(name="d", bufs=5))
        rpool = ctx.enter_context(tc.tile_pool(name="r", bufs=4))
        ndpool = ctx.enter_context(tc.tile_pool(name="nd", bufs=3))
        ps_d = ctx.enter_context(tc.tile_pool(name="ps_d", bufs=2,
                                              space="PSUM"))
        ps_cb = ctx.enter_context(tc.tile_pool(name="ps_cb", bufs=2,
                                               space="PSUM"))

        ident = cpool.tile([P, P], f16, tag="ident")
        make_identity(nc, ident[:])
        neg3 = cpool.tile([3, 1], f32, tag="neg3")
        nc.gpsimd.memset(neg3[:], -1.0)

        # ---- x^T parts of the source tables (DRAM->DRAM) ----
        nc.sync.dma_start(tsrc[1][:, 0:cs[3]], ft[3][:, :])
        nc.sync.dma_start(tsrc[2][:, 0:cs[2]], ft[2][:, :])
        nc.sync.dma_start(tsrc[3][:, 0:cs[1]], ft[1][:, :])

        # ---------- stages ----------
        for s, st in enumerate(stages):
            nf, S, D2 = st["nf"], st["S"], st["d2"]
            T = nf // P
            fine = 3 - s if s < 3 else 0
            coarse = 4 - s
            nchunk = (S + 511) // 512

            # K=21 bf16 triple-split distance operands (abs error ~1e-7,
            # needed because this data has near-coincident points across
            # levels): 2a = a1+a2+a3, b = b1+b2+b3, -|b|^2 = n1+n2+n3; the
            # six product pairs (11,12,21,13,31,22) plus three norm rows give
            # nd = 2a.b - |b|^2 at one PE cycle/row.
            bf32 = abpool.tile([3, S], f32, tag="bf32")
            nc.sync.dma_start(bf32[:], xt[coarse][:, :])
            sqb = abpool.tile([3, S], f32, tag="sqb")
            nc.vector.tensor_tensor(sqb[:], bf32[:], bf32[:], op=Alu.mult)
            for h0 in range(0, S, 1024):
                hw_ = min(1024, S - h0)
                pnb = ps_cb.tile([P, 1024], f32, tag="pcb")
                for c0 in range(0, hw_, 512):
                    w = min(512, hw_ - c0)
                    nc.tensor.matmul(pnb[0:1, c0:c0 + w], neg3[:],
                                     sqb[:, h0 + c0:h0 + c0 + w],
                                     start=True, stop=True)
                nc.scalar.copy(sqb[0:1, h0:h0 + hw_], pnb[0:1, :hw_])
            n1 = abpool.tile([1, S], bf16, tag="n1")
            nc.scalar.copy(n1[:], sqb[0:1, :])
            tn = abpool.tile([1, S], f32, tag="tn")
            nc.vector.tensor_tensor(tn[:], sqb[0:1, :], n1[:],
                                    op=Alu.subtract)
            n2 = abpool.tile([1, S], bf16, tag="n2")
            nc.scalar.copy(n2[:], tn[:])
            nc.vector.tensor_tensor(tn[:], tn[:], n2[:], op=Alu.subtract)
            n3 = abpool.tile([1, S], bf16, tag="n3")
            nc.scalar.copy(n3[:], tn[:])
            bs = bsp[coarse]
            b21 = abpool.tile([21, S], bf16, tag="b21")
            for r, j in enumerate([0, 1, 0, 2, 0, 1]):
                nc.sync.dma_start(b21[r * 3:(r + 1) * 3, :], bs[j][:, :])
            nc.sync.dma_start(b21[18:19, :], n1[:])
            nc.sync.dma_start(b21[19:20, :], n2[:])
            nc.sync.dma_start(b21[20:21, :], n3[:])

            a_s = asp[fine]
            a21 = abpool.tile([21, nf], bf16, tag="a21")
            for r, j in enumerate([0, 0, 1, 0, 2, 1]):
                nc.sync.dma_start(a21[r * 3:(r + 1) * 3, :], a_s[j][:, :])
            nc.sync.dma_start(a21[18:21, :], onesd[:, :nf])

            # query norms + eps, row layout: point t*128+p at [p, t]
            rx = smpool.tile([P, T * 3], f32, tag="rx")
            nc.sync.dma_start(
                rx[:].rearrange("p (t c) -> p t c", c=3),
                xr[fine][:, :].rearrange("(t p) c -> p t c", p=P))
            sqa = smpool.tile([P, T * 3], f32, tag="sqa")
            nc.vector.tensor_tensor(sqa[:], rx[:], rx[:], op=Alu.mult)
            nanrm = smpool.tile([P, T], f32, tag="nanrm")
            nc.vector.tensor_reduce(
                nanrm[:], sqa[:].rearrange("p (t c) -> p t c", c=3),
                axis=Axis.X, op=Alu.add)
            nc.vector.tensor_scalar_mul(nanrm[:], nanrm[:], -1.0)

            # per-stage arrays
            maxb = smpool.tile([P, T * 8], f32, tag="maxb")
            idxb = smpool.tile([P, T * 8], u32, tag="idxb")
            d3 = smpool.tile([P, T * KNN], f32, tag="d3")
            w3 = smpool.tile([P, T * KNN], f32, tag="w3")
            wgt = smpool.tile([P, T * KNN], f32, tag="wgt")
            wsum = smpool.tile([P, T], f32, tag="wsum")

            # software-pipelined tile loop: slot i runs the front half
            # (distances, top-3, gather launch, weights, diag) for tile i and
            # the back half (PE combine, psum copies, row write) for tile
            # i-LAG, so no engine's in-order stream sits behind work whose
            # inputs (the gathers) are still in flight.
            gts, diags = {}, {}
            for i in range(T + LAG):
                if i < T:
                    t = i
                    # nd_sb holds -d in fp16: the per-query |a|^2 is folded in
                    # as an ACT bias during the PSUM drain, so small distances
                    # keep full fp16 relative precision.
                    nd_sb = ndpool.tile([P, max(S, 512)], f16, tag="nd")
                    for h0 in range(0, S, 1024):
                        hw_ = min(1024, S - h0)
                        psd = ps_d.tile([P, 1024], f32, tag="psd")
                        for c0 in range(0, hw_, 512):
                            w = min(512, hw_ - c0)
                            nc.tensor.matmul(
                                psd[:, c0:c0 + w],
                                a21[:, t * P:(t + 1) * P],
                                b21[:, h0 + c0:h0 + c0 + w],
                                start=True, stop=True)
                        nc.scalar.activation(
                            nd_sb[:, h0:h0 + hw_], psd[:, :hw_],
                            mybir.ActivationFunctionType.Identity,
                            bias=nanrm[:, t:t + 1], scale=1.0)
                    nc.vector.max(maxb[:, t * 8:(t + 1) * 8], nd_sb[:, :S])
                    nc.vector.max_index(idxb[:, t * 8:(t + 1) * 8],
                                        maxb[:, t * 8:(t + 1) * 8],
                                        nd_sb[:, :S])
                    gt = gpool.tile([P, KNN * D2], f16, tag="gt")
                    gts[t] = gt
                    for k in range(KNN):
                        nc.gpsimd.indirect_dma_start(
                            out=gt[:, k * D2:(k + 1) * D2], out_offset=None,
                            in_=gsrc[s][:, :],
                            in_offset=IndirectOffsetOnAxis(
                                ap=idxb[:, t * 8 + k:t * 8 + k + 1], axis=0))
                    # per-tile weights: maxb = -d, so d3 = -maxb + eps
                    t3 = slice(t * KNN, (t + 1) * KNN)
                    nc.vector.tensor_scalar(
                        d3[:, t3], maxb[:, t * 8:t * 8 + KNN], -1.0,
                        EPS, op0=Alu.mult, op1=Alu.add)
                    nc.vector.tensor_scalar_max(d3[:, t3], d3[:, t3], DFLOOR)
                    nc.vector.reciprocal(w3[:, t3], d3[:, t3])
                    nc.vector.tensor_reduce(
                        wsum[:, t:t + 1],
                        w3[:, t3].rearrange("p (o e) -> p o e", o=1),
                        axis=Axis.X, op=Alu.add)
                    nc.vector.reciprocal(wsum[:, t:t + 1], wsum[:, t:t + 1])
                    nc.vector.tensor_scalar_mul(wgt[:, t3], w3[:, t3],
                                                wsum[:, t:t + 1])
                    diag = dpool.tile([P, KNN * P], f16, tag="diag")
                    diags[t] = diag
                    for k in range(KNN):
                        nc.vector.tensor_scalar_mul(
                            diag[:, k * P:(k + 1) * P], ident[:],
                            wgt[:, t * KNN + k:t * KNN + k + 1])
                if i >= LAG:
                    t = i - LAG
                    gt, diag = gts.pop(t), diags.pop(t)
                    res = rpool.tile([P, D2], f16, tag="res")
                    for h0 in range(0, D2, 1024):
                        hw_ = min(1024, D2 - h0)
                        pcb = ps_cb.tile([P, 1024], f32, tag="pcb")
                        for c0 in range(0, hw_, 512):
                            w = min(512, hw_ - c0)
                            for k in range(KNN):
                                nc.tensor.matmul(
                                    pcb[:, c0:c0 + w],
                                    diag[:, k * P:(k + 1) * P],
                                    gt[:, k * D2 + h0 + c0:
                                       k * D2 + h0 + c0 + w],
                                    start=(k == 0), stop=(k == KNN - 1))
                        nc.scalar.copy(res[:, h0:h0 + hw_], pcb[:, :hw_])
                    if s < 3:
                        nc.sync.dma_start(
                            tsrc[s + 1][t * P:(t + 1) * P,
                                        st["cx"]:st["cx"] + D2], res[:])
                    else:
                        nc.sync.dma_start(oi[t * P:(t + 1) * P, :], res[:])
    if split_waits:
        _split_multi_waits(nc)
    return nc


def _split_multi_waits(nc):
    """This walrus build rejects instructions carrying more than one sync
    wait. Hoist extra waits into same-engine NoOps inserted just before."""
    import concourse.mybir as mybir

    n = 0
    for f in nc.m.functions:
        for bb in f.blocks:
            il = bb.instructions
            i = 0
            while i < len(il):
                inst = il[i]
                si = getattr(inst, "sync_info", None)
                ow = list(si.on_wait) if si is not None else []
                if len(ow) > 1:
                    for w in ow[:-1]:
                        nop = mybir.InstNoOp(name=f"W{n}-{inst.name}",
                                             ins=[], outs=[])
                        n += 1
                        nop.engine = inst.engine
                        nop.sync_info = mybir.SyncInfo(on_update=[],
                                                       on_wait=[w])
                        il.insert(i, nop)
                        i += 1
                    inst.sync_info = mybir.SyncInfo(
                        on_update=list(si.on_update), on_wait=[ow[-1]])
                i += 1


def _patch_tile_drain():
    """This walrus build rejects >1 sync-wait on the kernel-tail Drain; spread
    the waits across single-wait SP nops instead."""
    import concourse.mybir as mybir
    import concourse.tile as tile
    from concourse.vector_clock import ScopedClock

    if getattr(tile.TileContext, "_drain_patched", False):
        return

    def _patched(self, tick_clock, wait_clock):
        nc = self.nc
        probe = nc.sync.nop()
        wait_clock.add_sem_waits(probe.ins,
                                 ScopedClock({None: tick_clock.global_clock}))
        si = probe.ins.sync_info
        ow = list(si.on_wait) if si is not None else []
        if len(ow) > 1:
            for w in ow[1:]:
                n2 = nc.sync.nop()
                n2.ins.sync_info = mybir.SyncInfo(on_update=[], on_wait=[w])
            probe.ins.sync_info = mybir.SyncInfo(on_update=list(si.on_update),
                                                 on_wait=[ow[0]])
        nc.sync.drain()
        nc.all_engine_barrier()
        assert self.sems is not None
        popped = nc._tile_sem_poison_stack.pop()
        assert popped is self._sem_poison
        nc.clear_and_free_semaphores(list(self.sems.allocated().values()))
        nc.all_engine_barrier()

    tile.TileContext._drain_and_barrier = _patched
    tile.TileContext._drain_patched = True


def _get_program(ns, cs, n_half):
    key = (tuple(ns), tuple(cs), n_half)
    if _CACHED["key"] != key:
        _CACHED["nc"] = _build_program(ns, cs, n_half)
        _CACHED["key"] = key
    return _CACHED["nc"]


def _split3(x):
    """Three-way bf16 split of an f32 array: x ~ s0+s1+s2 to ~2^-27."""
    import ml_dtypes
    s0 = x.astype(ml_dtypes.bfloat16)
    r = x - s0.astype(np.float32)
    s1 = r.astype(ml_dtypes.bfloat16)
    r = r - s1.astype(np.float32)
    s2 = r.astype(ml_dtypes.bfloat16)
    return [np.ascontiguousarray(s) for s in (s0, s1, s2)]


def make_core_inputs(inputs, ns, n_half, core):
    """Slice/transform full inputs for one core (b = core//2, half = core%2)."""
    import ml_dtypes
    b, h = core // 2, core % 2
    d = {}
    x0h = np.ascontiguousarray(
        np.asarray(inputs["xyz0"])[b, h * n_half:(h + 1) * n_half])
    d["xt0"] = np.ascontiguousarray(x0h.T)
    d["xr0"] = x0h
    for j, s in enumerate(_split3(2.0 * d["xt0"])):
        d[f"a{j}_0"] = s
    for i in range(1, 5):
        xi = np.ascontiguousarray(np.asarray(inputs[f"xyz{i}"])[b])
        d[f"xt{i}"] = np.ascontiguousarray(xi.T)
        d[f"xr{i}"] = xi
        for j, s in enumerate(_split3(2.0 * d[f"xt{i}"])):
            d[f"a{j}_{i}"] = s
        for j, s in enumerate(_split3(d[f"xt{i}"])):
            d[f"b{j}_{i}"] = s
        d[f"ft{i}"] = np.ascontiguousarray(
            np.asarray(inputs[f"x{i}"])[b].T.astype(np.float16))
    d["ones"] = np.ones((3, max(n_half, ns[1])), ml_dtypes.bfloat16)
    return d


def kernel(**inputs):
    from concourse.bass_utils import run_bass_kernel_spmd

    ns, cs = NS, CS
    n_half = ns[0] // 2
    nc = _get_program(ns, cs, n_half)

    in_maps = [make_core_inputs(inputs, ns, n_half, c) for c in range(8)]
    res = run_bass_kernel_spmd(nc, in_maps, core_ids=list(range(8)))

    dout = sum(cs)
    out = np.empty((B, dout, ns[0]), np.float32)
    out[:, :cs[0], :] = np.asarray(inputs["x0"])
    for c in range(8):
        b, h = c // 2, c % 2
        out[b, cs[0]:, h * n_half:(h + 1) * n_half] = \
            res.results[c]["oi"].astype(np.float32).T
    return out


# revision 21
# speedup vs baseline: 1.1021x; 1.1021x over previous
"""Trainium2 Bass kernel for a PointNet++-style feature-propagation decoder
(4 stages of kNN(k=3) inverse-distance-weighted feature interpolation).

Sharding: batch b = core//2 (data parallel over B=4); the finest stage's 8192
query points split in half across each core pair (point parallel along N per
the sharding hint). Stages 0-2 are duplicated within a pair; stage 3 is
n-split. Output rows 0:64 are the x0 passthrough, assembled on the host.

Per-core device pipeline per stage (all feature traffic in fp16):
  - negated partial distances nd = 2a.b - |b|^2 via one K=4 fp32 PE matmul
    per 128-query tile (the -|a|^2 term is constant per query and cannot
    change each query's top-k order; it is restored only for the 3 selected
    values when computing weights). |b|^2 rows come from a tiny ones-matmul.
  - top-3 neighbors via DVE max (top-8) + max_index read directly off PSUM
  - inverse-distance weights on DVE (batched per 8-tile block)
  - feature gather via SWDGE indirect DMA from fp16 DRAM tables
  - weighted 3-way combine on the PE: psum += diag(w_k) @ gathered_k, where
    diag(w_k) = identity * w_k built by one 4x-mode DVE tensor_scalar each
  - stage results are written as fp16 table rows for the next stage's gather;
    the final stage writes fp16 [n, d] rows that the host transposes.
"""

import numpy as np

P = 128
KNN = 3
EPS = 1e-8
DFLOOR = 1e-7
LAG = 3

B = 4
NS = [8192, 2048, 512, 128, 32]  # points per level, finest -> coarsest
CS = [64, 128, 256, 512, 1024]   # feature channels per level

_CACHED = {"nc": None, "key": None}


def _build_program(ns, cs, n_half, split_waits=True):
    """Trace the per-core Bass program. ns/cs as in reference (finest first).
    n_half: number of finest-level query points this core handles."""
    import contextlib

    import concourse.bass as bass
    import concourse.mybir as mybir
    import concourse.tile as tile
    from concourse.bass import IndirectOffsetOnAxis
    from concourse.masks import make_identity

    _patch_tile_drain()

    f32 = mybir.dt.float32
    f16 = mybir.dt.float16
    bf16 = mybir.dt.bfloat16
    u32 = mybir.dt.uint32
    Alu = mybir.AluOpType
    Axis = mybir.AxisListType

    # stage s: fine level (3-s) [this core's half of level 0 for s=3],
    # coarse level (4-s). d2[s]: width of the table gathered at stage s.
    d2 = [cs[4]]
    for s in range(1, 4):
        d2.append(cs[4 - s] + d2[s - 1])
    stages = []
    for s in range(4):
        nf = ns[3 - s] if s < 3 else n_half
        stages.append(dict(nf=nf, S=ns[4 - s], d2=d2[s],
                           cx=(cs[3 - s] if s < 3 else None)))

    nc = bass.Bass("TRN2")

    # ---- external inputs (per core) ----
    xt = {}   # xyz transposed [3, N] f32
    xr = {}   # xyz rows [N, 3] f32
    asp, bsp = {}, {}
    for i, n in enumerate(ns):
        nn_ = n_half if i == 0 else n
        xt[i] = nc.dram_tensor(f"xt{i}", [3, nn_], f32, kind="ExternalInput")
        xr[i] = nc.dram_tensor(f"xr{i}", [nn_, 3], f32, kind="ExternalInput")
        asp[i] = [nc.dram_tensor(f"a{j}_{i}", [3, nn_], bf16,
                                 kind="ExternalInput") for j in range(3)]
        if i > 0:
            bsp[i] = [nc.dram_tensor(f"b{j}_{i}", [3, n], bf16,
                                     kind="ExternalInput") for j in range(3)]
    ft = {}   # features transposed [N, C] fp16
    for i in range(1, 5):
        ft[i] = nc.dram_tensor(f"ft{i}", [ns[i], cs[i]], f16,
                               kind="ExternalInput")

    onesd = nc.dram_tensor("ones", [3, max(n_half, ns[1])], bf16,
                           kind="ExternalInput")

    # ---- external output: interp part of final stage, fp16 rows ----
    oi = nc.dram_tensor("oi", [n_half, d2[3]], f16, kind="ExternalOutput")

    # ---- internal fp16 DRAM source tables (stage 0 gathers ft[4]) ----
    tsrc = {s: nc.dram_tensor(f"tsrc{s}", [stages[s]["S"], stages[s]["d2"]],
                              f16)
            for s in range(1, 4)}
    gsrc = {0: ft[4], 1: tsrc[1], 2: tsrc[2], 3: tsrc[3]}

    with tile.TileContext(nc) as tc, contextlib.ExitStack() as ctx:
        cpool = ctx.enter_context(tc.tile_pool(name="const", bufs=1))
        abpool = ctx.enter_context(tc.tile_pool(name="ab", bufs=2))
        smpool = ctx.enter_context(tc.tile_pool(name="sm", bufs=2))
        gpool = ctx.enter_context(tc.tile_pool(name="g", bufs=5))
        dpool = ctx.enter_context(tc.tile_pool(name="d", bufs=5))
        rpool = ctx.enter_context(tc.tile_pool(name="r", bufs=4))
        ndpool = ctx.enter_context(tc.tile_pool(name="nd", bufs=3))
        ps_d = ctx.enter_context(tc.tile_pool(name="ps_d", bufs=2,
                                              space="PSUM"))
        ps_cb = ctx.enter_context(tc.tile_pool(name="ps_cb", bufs=2,
                                               space="PSUM"))

        ident = cpool.tile([P, P], f16, tag="ident")
        make_identity(nc, ident[:])
        neg3 = cpool.tile([3, 1], f32, tag="neg3")
        nc.gpsimd.memset(neg3[:], -1.0)

        # ---- x^T parts of the source tables (DRAM->DRAM) ----
        nc.sync.dma_start(tsrc[1][:, 0:cs[3]], ft[3][:, :])
        nc.sync.dma_start(tsrc[2][:, 0:cs[2]], ft[2][:, :])
        nc.sync.dma_start(tsrc[3][:, 0:cs[1]], ft[1][:, :])

        # ---------- stages ----------
        # Setup for stage s+1 is emitted before stage s's tile loop so the
        # operand assembly (DMAs, norm matmuls, splits) overlaps the previous
        # stage's compute instead of serializing at the boundary.
        def emit_setup(s):
            st = stages[s]
            nf, S = st["nf"], st["S"]
            T = nf // P
            fine = 3 - s if s < 3 else 0
            coarse = 4 - s

            # K=21 bf16 triple-split distance operands (abs error ~1e-7,
            # needed because this data has near-coincident points across
            # levels): 2a = a1+a2+a3, b = b1+b2+b3, -|b|^2 = n1+n2+n3; the
            # six product pairs (11,12,21,13,31,22) plus three norm rows give
            # nd = 2a.b - |b|^2 at one PE cycle/row.
            bf32 = abpool.tile([3, S], f32, tag="bf32")
            nc.sync.dma_start(bf32[:], xt[coarse][:, :])
            sqb = abpool.tile([3, S], f32, tag="sqb")
            nc.vector.tensor_tensor(sqb[:], bf32[:], bf32[:], op=Alu.mult)
            for h0 in range(0, S, 1024):
                hw_ = min(1024, S - h0)
                pnb = ps_cb.tile([P, 1024], f32, tag="pcb")
                for c0 in range(0, hw_, 512):
                    w = min(512, hw_ - c0)
                    nc.tensor.matmul(pnb[0:1, c0:c0 + w], neg3[:],
                                     sqb[:, h0 + c0:h0 + c0 + w],
                                     start=True, stop=True)
                nc.scalar.copy(sqb[0:1, h0:h0 + hw_], pnb[0:1, :hw_])
            n1 = abpool.tile([1, S], bf16, tag="n1")
            nc.scalar.copy(n1[:], sqb[0:1, :])
            tn = abpool.tile([1, S], f32, tag="tn")
            nc.vector.tensor_tensor(tn[:], sqb[0:1, :], n1[:],
                                    op=Alu.subtract)
            n2 = abpool.tile([1, S], bf16, tag="n2")
            nc.scalar.copy(n2[:], tn[:])
            nc.vector.tensor_tensor(tn[:], tn[:], n2[:], op=Alu.subtract)
            n3 = abpool.tile([1, S], bf16, tag="n3")
            nc.scalar.copy(n3[:], tn[:])
            bs = bsp[coarse]
            b21 = abpool.tile([21, S], bf16, tag="b21")
            for r, j in enumerate([0, 1, 0, 2, 0, 1]):
                nc.sync.dma_start(b21[r * 3:(r + 1) * 3, :], bs[j][:, :])
            nc.sync.dma_start(b21[18:19, :], n1[:])
            nc.sync.dma_start(b21[19:20, :], n2[:])
            nc.sync.dma_start(b21[20:21, :], n3[:])

            a_s = asp[fine]
            a21 = abpool.tile([21, nf], bf16, tag="a21")
            for r, j in enumerate([0, 0, 1, 0, 2, 1]):
                nc.sync.dma_start(a21[r * 3:(r + 1) * 3, :], a_s[j][:, :])
            nc.sync.dma_start(a21[18:21, :], onesd[:, :nf])

            # query norms + eps, row layout: point t*128+p at [p, t]
            rx = smpool.tile([P, T * 3], f32, tag="rx")
            nc.sync.dma_start(
                rx[:].rearrange("p (t c) -> p t c", c=3),
                xr[fine][:, :].rearrange("(t p) c -> p t c", p=P))
            sqa = smpool.tile([P, T * 3], f32, tag="sqa")
            nc.vector.tensor_tensor(sqa[:], rx[:], rx[:], op=Alu.mult)
            nanrm = smpool.tile([P, T], f32, tag="nanrm")
            nc.vector.tensor_reduce(
                nanrm[:], sqa[:].rearrange("p (t c) -> p t c", c=3),
                axis=Axis.X, op=Alu.add)
            nc.vector.tensor_scalar_mul(nanrm[:], nanrm[:], -1.0)

            # per-stage arrays
            maxb = smpool.tile([P, T * 8], f32, tag="maxb")
            idxb = smpool.tile([P, T * 8], u32, tag="idxb")
            d3 = smpool.tile([P, T * KNN], f32, tag="d3")
            w3 = smpool.tile([P, T * KNN], f32, tag="w3")
            wgt = smpool.tile([P, T * KNN], f32, tag="wgt")
            wsum = smpool.tile([P, T], f32, tag="wsum")
            return dict(a21=a21, b21=b21, nanrm=nanrm, maxb=maxb, idxb=idxb,
                        d3=d3, w3=w3, wgt=wgt, wsum=wsum)

        def emit_loop(s, cx_):
            st = stages[s]
            nf, S, D2 = st["nf"], st["S"], st["d2"]
            T = nf // P
            a21, b21, nanrm = cx_["a21"], cx_["b21"], cx_["nanrm"]
            maxb, idxb = cx_["maxb"], cx_["idxb"]
            d3, w3, wgt, wsum = cx_["d3"], cx_["w3"], cx_["wgt"], cx_["wsum"]

            # software-pipelined tile loop: slot i runs the front half
            # (distances, top-3, gather launch, weights, diag) for tile i and
            # the back half (PE combine, psum copies, row write) for tile
            # i-LAG, so no engine's in-order stream sits behind work whose
            # inputs (the gathers) are still in flight.
            gts, diags = {}, {}
            for i in range(T + LAG):
                if i < T:
                    t = i
                    # nd_sb holds -d in fp16: the per-query |a|^2 is folded in
                    # as an ACT bias during the PSUM drain, so small distances
                    # keep full fp16 relative precision.
                    nd_sb = ndpool.tile([P, max(S, 512)], f16, tag="nd")
                    for h0 in range(0, S, 1024):
                        hw_ = min(1024, S - h0)
                        psd = ps_d.tile([P, 1024], f32, tag="psd")
                        for c0 in range(0, hw_, 512):
                            w = min(512, hw_ - c0)
                            nc.tensor.matmul(
                                psd[:, c0:c0 + w],
                                a21[:, t * P:(t + 1) * P],
                                b21[:, h0 + c0:h0 + c0 + w],
                                start=True, stop=True)
                        nc.scalar.activation(
                            nd_sb[:, h0:h0 + hw_], psd[:, :hw_],
                            mybir.ActivationFunctionType.Identity,
                            bias=nanrm[:, t:t + 1], scale=1.0)
                    nc.vector.max(maxb[:, t * 8:(t + 1) * 8], nd_sb[:, :S])
                    nc.vector.max_index(idxb[:, t * 8:(t + 1) * 8],
                                        maxb[:, t * 8:(t + 1) * 8],
                                        nd_sb[:, :S])
                    gt = gpool.tile([P, KNN * D2], f16, tag="gt")
                    gts[t] = gt
                    for k in range(KNN):
                        nc.gpsimd.indirect_dma_start(
                            out=gt[:, k * D2:(k + 1) * D2], out_offset=None,
                            in_=gsrc[s][:, :],
                            in_offset=IndirectOffsetOnAxis(
                                ap=idxb[:, t * 8 + k:t * 8 + k + 1], axis=0))
                    # per-tile weights: maxb = -d, so d3 = -maxb + eps
                    t3 = slice(t * KNN, (t + 1) * KNN)
                    nc.vector.tensor_scalar(
                        d3[:, t3], maxb[:, t * 8:t * 8 + KNN], -1.0,
                        EPS, op0=Alu.mult, op1=Alu.add)
                    nc.vector.tensor_scalar_max(d3[:, t3], d3[:, t3], DFLOOR)
                    nc.vector.reciprocal(w3[:, t3], d3[:, t3])
                    nc.vector.tensor_reduce(
                        wsum[:, t:t + 1],
                        w3[:, t3].rearrange("p (o e) -> p o e", o=1),
                        axis=Axis.X, op=Alu.add)
                    nc.vector.reciprocal(wsum[:, t:t + 1], wsum[:, t:t + 1])
                    nc.vector.tensor_scalar_mul(wgt[:, t3], w3[:, t3],
                                                wsum[:, t:t + 1])
                    diag = dpool.tile([P, KNN * P], f16, tag="diag")
                    diags[t] = diag
                    for k in range(KNN):
                        nc.vector.tensor_scalar_mul(
                            diag[:, k * P:(k + 1) * P], ident[:],
                            wgt[:, t * KNN + k:t * KNN + k + 1])
                if i >= LAG:
                    t = i - LAG
                    gt, diag = gts.pop(t), diags.pop(t)
                    res = rpool.tile([P, D2], f16, tag="res")
                    for h0 in range(0, D2, 1024):
                        hw_ = min(1024, D2 - h0)
                        pcb = ps_cb.tile([P, 1024], f32, tag="pcb")
                        for c0 in range(0, hw_, 512):
                            w = min(512, hw_ - c0)
                            for k in range(KNN):
                                nc.tensor.matmul(
                                    pcb[:, c0:c0 + w],
                                    diag[:, k * P:(k + 1) * P],
                                    gt[:, k * D2 + h0 + c0:
                                       k * D2 + h0 + c0 + w],
                                    start=(k == 0), stop=(k == KNN - 1))
                        nc.scalar.copy(res[:, h0:h0 + hw_], pcb[:, :hw_])
                    if s < 3:
                        nc.sync.dma_start(
                            tsrc[s + 1][t * P:(t + 1) * P,
                                        st["cx"]:st["cx"] + D2], res[:])
                    else:
                        nc.sync.dma_start(oi[t * P:(t + 1) * P, :], res[:])

        cx0 = emit_setup(0)
        cx1 = emit_setup(1)
        emit_loop(0, cx0)
        cx2 = emit_setup(2)
        emit_loop(1, cx1)
        cx3 = emit_setup(3)
        emit_loop(2, cx2)
        emit_loop(3, cx3)
    if split_waits:
        _split_multi_waits(nc)
    return nc


def _split_multi_waits(nc):
    """This walrus build rejects instructions carrying more than one sync
    wait. Hoist extra waits into same-engine NoOps inserted just before."""
    import concourse.mybir as mybir

    n = 0
    for f in nc.m.functions:
        for bb in f.blocks:
            il = bb.instructions
            i = 0
            while i < len(il):
                inst = il[i]
                si = getattr(inst, "sync_info", None)
                ow = list(si.on_wait) if si is not None else []
                if len(ow) > 1:
                    for w in ow[:-1]:
                        nop = mybir.InstNoOp(name=f"W{n}-{inst.name}",
                                             ins=[], outs=[])
                        n += 1
                        nop.engine = inst.engine
                        nop.sync_info = mybir.SyncInfo(on_update=[],
                                                       on_wait=[w])
                        il.insert(i, nop)
                        i += 1
                    inst.sync_info = mybir.SyncInfo(
                        on_update=list(si.on_update), on_wait=[ow[-1]])
                i += 1


def _patch_tile_drain():
    """This walrus build rejects >1 sync-wait on the kernel-tail Drain; spread
    the waits across single-wait SP nops instead."""
    import concourse.mybir as mybir
    import concourse.tile as tile
    from concourse.vector_clock import ScopedClock

    if getattr(tile.TileContext, "_drain_patched", False):
        return

    def _patched(self, tick_clock, wait_clock):
        nc = self.nc
        probe = nc.sync.nop()
        wait_clock.add_sem_waits(probe.ins,
                                 ScopedClock({None: tick_clock.global_clock}))
        si = probe.ins.sync_info
        ow = list(si.on_wait) if si is not None else []
        if len(ow) > 1:
            for w in ow[1:]:
                n2 = nc.sync.nop()
                n2.ins.sync_info = mybir.SyncInfo(on_update=[], on_wait=[w])
            probe.ins.sync_info = mybir.SyncInfo(on_update=list(si.on_update),
                                                 on_wait=[ow[0]])
        nc.sync.drain()
        nc.all_engine_barrier()
        assert self.sems is not None
        popped = nc._tile_sem_poison_stack.pop()
        assert popped is self._sem_poison
        nc.clear_and_free_semaphores(list(self.sems.allocated().values()))
        nc.all_engine_barrier()

    tile.TileContext._drain_and_barrier = _patched
    tile.TileContext._drain_patched = True


def _get_program(ns, cs, n_half):
    key = (tuple(ns), tuple(cs), n_half)
    if _CACHED["key"] != key:
        _CACHED["nc"] = _build_program(ns, cs, n_half)
        _CACHED["key"] = key
    return _CACHED["nc"]


def _split3(x):
    """Three-way bf16 split of an f32 array: x ~ s0+s1+s2 to ~2^-27."""
    import ml_dtypes
    s0 = x.astype(ml_dtypes.bfloat16)
    r = x - s0.astype(np.float32)
    s1 = r.astype(ml_dtypes.bfloat16)
    r = r - s1.astype(np.float32)
    s2 = r.astype(ml_dtypes.bfloat16)
    return [np.ascontiguousarray(s) for s in (s0, s1, s2)]


def make_core_inputs(inputs, ns, n_half, core):
    """Slice/transform full inputs for one core (b = core//2, half = core%2)."""
    import ml_dtypes
    b, h = core // 2, core % 2
    d = {}
    x0h = np.ascontiguousarray(
        np.asarray(inputs["xyz0"])[b, h * n_half:(h + 1) * n_half])
    d["xt0"] = np.ascontiguousarray(x0h.T)
    d["xr0"] = x0h
    for j, s in enumerate(_split3(2.0 * d["xt0"])):
        d[f"a{j}_0"] = s
    for i in range(1, 5):
        xi = np.ascontiguousarray(np.asarray(inputs[f"xyz{i}"])[b])
        d[f"xt{i}"] = np.ascontiguousarray(xi.T)
        d[f"xr{i}"] = xi
        for j, s in enumerate(_split3(2.0 * d[f"xt{i}"])):
            d[f"a{j}_{i}"] = s
        for j, s in enumerate(_split3(d[f"xt{i}"])):
            d[f"b{j}_{i}"] = s
        d[f"ft{i}"] = np.ascontiguousarray(
            np.asarray(inputs[f"x{i}"])[b].T.astype(np.float16))
    d["ones"] = np.ones((3, max(n_half, ns[1])), ml_dtypes.bfloat16)
    return d


def kernel(**inputs):
    from concourse.bass_utils import run_bass_kernel_spmd

    ns, cs = NS, CS
    n_half = ns[0] // 2
    nc = _get_program(ns, cs, n_half)

    in_maps = [make_core_inputs(inputs, ns, n_half, c) for c in range(8)]
    res = run_bass_kernel_spmd(nc, in_maps, core_ids=list(range(8)))

    dout = sum(cs)
    out = np.empty((B, dout, ns[0]), np.float32)
    out[:, :cs[0], :] = np.asarray(inputs["x0"])
    for c in range(8):
        b, h = c // 2, c % 2
        out[b, cs[0]:, h * n_half:(h + 1) * n_half] = \
            res.results[c]["oi"].astype(np.float32).T
    return out


# revision 25
# speedup vs baseline: 1.1258x; 1.0216x over previous
"""Trainium2 Bass kernel for a PointNet++-style feature-propagation decoder
(4 stages of kNN(k=3) inverse-distance-weighted feature interpolation).

Sharding: batch b = core//2 (data parallel over B=4); the finest stage's 8192
query points split in half across each core pair (point parallel along N per
the sharding hint). Stages 0-2 are duplicated within a pair; stage 3 is
n-split. Output rows 0:64 are the x0 passthrough, assembled on the host.

Per-core device pipeline per stage (all feature traffic in fp16):
  - negated partial distances nd = 2a.b - |b|^2 via one K=4 fp32 PE matmul
    per 128-query tile (the -|a|^2 term is constant per query and cannot
    change each query's top-k order; it is restored only for the 3 selected
    values when computing weights). |b|^2 rows come from a tiny ones-matmul.
  - top-3 neighbors via DVE max (top-8) + max_index read directly off PSUM
  - inverse-distance weights on DVE (batched per 8-tile block)
  - feature gather via SWDGE indirect DMA from fp16 DRAM tables
  - weighted 3-way combine on the PE: psum += diag(w_k) @ gathered_k, where
    diag(w_k) = identity * w_k built by one 4x-mode DVE tensor_scalar each
  - stage results are written as fp16 table rows for the next stage's gather;
    the final stage writes fp16 [n, d] rows that the host transposes.
"""

import numpy as np

P = 128
KNN = 3
EPS = 1e-8
DFLOOR = 1e-7
LAG = 3

B = 4
NS = [8192, 2048, 512, 128, 32]  # points per level, finest -> coarsest
CS = [64, 128, 256, 512, 1024]   # feature channels per level

_CACHED = {"nc": None, "key": None}


def _build_program(ns, cs, n_half, split_waits=True):
    """Trace the per-core Bass program. ns/cs as in reference (finest first).
    n_half: number of finest-level query points this core handles."""
    import contextlib

    import concourse.bass as bass
    import concourse.mybir as mybir
    import concourse.tile as tile
    from concourse.bass import IndirectOffsetOnAxis
    from concourse.masks import make_identity

    _patch_tile_drain()

    f32 = mybir.dt.float32
    f16 = mybir.dt.float16
    bf16 = mybir.dt.bfloat16
    u32 = mybir.dt.uint32
    Alu = mybir.AluOpType
    Axis = mybir.AxisListType

    # stage s: fine level (3-s) [this core's half of level 0 for s=3],
    # coarse level (4-s). d2[s]: width of the table gathered at stage s.
    d2 = [cs[4]]
    for s in range(1, 4):
        d2.append(cs[4 - s] + d2[s - 1])
    stages = []
    for s in range(4):
        nf = ns[3 - s] if s < 3 else n_half
        stages.append(dict(nf=nf, S=ns[4 - s], d2=d2[s],
                           cx=(cs[3 - s] if s < 3 else None)))

    nc = bass.Bass("TRN2")

    # ---- external inputs (per core) ----
    xt = {}   # xyz transposed [3, N] f32
    xr = {}   # xyz rows [N, 3] f32
    asp, bsp = {}, {}
    for i, n in enumerate(ns):
        nn_ = n_half if i == 0 else n
        xt[i] = nc.dram_tensor(f"xt{i}", [3, nn_], f32, kind="ExternalInput")
        xr[i] = nc.dram_tensor(f"xr{i}", [nn_, 3], f32, kind="ExternalInput")
        asp[i] = [nc.dram_tensor(f"a{j}_{i}", [3, nn_], bf16,
                                 kind="ExternalInput") for j in range(3)]
        if i > 0:
            bsp[i] = [nc.dram_tensor(f"b{j}_{i}", [3, n], bf16,
                                     kind="ExternalInput") for j in range(3)]
    ft = {}   # features transposed [N, C] fp16
    for i in range(1, 5):
        ft[i] = nc.dram_tensor(f"ft{i}", [ns[i], cs[i]], f16,
                               kind="ExternalInput")

    onesd = nc.dram_tensor("ones", [3, max(n_half, ns[1])], bf16,
                           kind="ExternalInput")

    # ---- external output: interp part of final stage, fp16 rows ----
    oi = nc.dram_tensor("oi", [n_half, d2[3]], f16, kind="ExternalOutput")

    # ---- internal fp16 DRAM source tables (stage 0 gathers ft[4]) ----
    tsrc = {3: nc.dram_tensor("tsrc3", [stages[3]["S"], stages[3]["d2"]],
                              f16)}
    gsrc = {3: tsrc[3]}

    with tile.TileContext(nc) as tc, contextlib.ExitStack() as ctx:
        cpool = ctx.enter_context(tc.tile_pool(name="const", bufs=1))
        abpool = ctx.enter_context(tc.tile_pool(name="ab", bufs=2))
        aspool = ctx.enter_context(tc.tile_pool(name="as", bufs=1))
        smpool = ctx.enter_context(tc.tile_pool(name="sm", bufs=2))
        gpool = ctx.enter_context(tc.tile_pool(name="g", bufs=4))
        dpool = ctx.enter_context(tc.tile_pool(name="d", bufs=5))
        rpool = ctx.enter_context(tc.tile_pool(name="r", bufs=4))
        ndpool = ctx.enter_context(tc.tile_pool(name="nd", bufs=3))
        wpool = ctx.enter_context(tc.tile_pool(name="w", bufs=2))
        wtpool = ctx.enter_context(tc.tile_pool(name="wt", bufs=5))
        ps_d = ctx.enter_context(tc.tile_pool(name="ps_d", bufs=1,
                                              space="PSUM"))
        ps_t = ctx.enter_context(tc.tile_pool(name="ps_t", bufs=2,
                                              space="PSUM"))
        ps_cb = ctx.enter_context(tc.tile_pool(name="ps_cb", bufs=2,
                                               space="PSUM"))

        ident = cpool.tile([P, P], f16, tag="ident")
        make_identity(nc, ident[:])
        neg3 = cpool.tile([3, 1], f32, tag="neg3")
        nc.gpsimd.memset(neg3[:], -1.0)
        iota16 = cpool.tile([P, 512], f16, tag="iota16")
        nc.gpsimd.iota(iota16[:], pattern=[[1, 512]], base=0,
                       channel_multiplier=0,
                       allow_small_or_imprecise_dtypes=True)

        # SBUF-resident source tables for the dense stages 0-2.
        # t2sb holds T2's four 128-row partition blocks side by side.
        t0sb = cpool.tile([ns[4], cs[4]], f16, tag="t0sb")
        nc.sync.dma_start(t0sb[:], ft[4][:, :])
        t1sb = cpool.tile([P, 1536], f16, tag="t1sb")
        nc.sync.dma_start(t1sb[:, 0:cs[3]], ft[3][:, :])
        t2sb = cpool.tile([P, 4 * 1792], f16, tag="t2sb")
        for blk in range(4):
            nc.sync.dma_start(t2sb[:, blk * 1792:blk * 1792 + cs[2]],
                              ft[2][blk * P:(blk + 1) * P, :])

        # x1 part of T3 in DRAM (stage 3 still gathers)
        nc.sync.dma_start(tsrc[3][:, 0:cs[1]], ft[1][:, :])

        # ---------- stages ----------
        # Setup for stage s+1 is emitted before stage s's tile loop so the
        # operand assembly (DMAs, norm matmuls, splits) overlaps the previous
        # stage's compute instead of serializing at the boundary.
        def emit_setup(s):
            st = stages[s]
            nf, S = st["nf"], st["S"]
            T = nf // P
            fine = 3 - s if s < 3 else 0
            coarse = 4 - s

            # K=21 bf16 triple-split distance operands (abs error ~1e-7,
            # needed because this data has near-coincident points across
            # levels): 2a = a1+a2+a3, b = b1+b2+b3, -|b|^2 = n1+n2+n3; the
            # six product pairs (11,12,21,13,31,22) plus three norm rows give
            # nd = 2a.b - |b|^2 at one PE cycle/row.
            bf32 = aspool.tile([3, S], f32, tag="bf32")
            nc.sync.dma_start(bf32[:], xt[coarse][:, :])
            sqb = aspool.tile([3, S], f32, tag="sqb")
            nc.vector.tensor_tensor(sqb[:], bf32[:], bf32[:], op=Alu.mult)
            for h0 in range(0, S, 1024):
                hw_ = min(1024, S - h0)
                pnb = ps_cb.tile([P, 1024], f32, tag="pcb")
                for c0 in range(0, hw_, 512):
                    w = min(512, hw_ - c0)
                    nc.tensor.matmul(pnb[0:1, c0:c0 + w], neg3[:],
                                     sqb[:, h0 + c0:h0 + c0 + w],
                                     start=True, stop=True)
                nc.scalar.copy(sqb[0:1, h0:h0 + hw_], pnb[0:1, :hw_])
            n1 = aspool.tile([1, S], bf16, tag="n1")
            nc.scalar.copy(n1[:], sqb[0:1, :])
            tn = aspool.tile([1, S], f32, tag="tn")
            nc.vector.tensor_tensor(tn[:], sqb[0:1, :], n1[:],
                                    op=Alu.subtract)
            n2 = aspool.tile([1, S], bf16, tag="n2")
            nc.scalar.copy(n2[:], tn[:])
            nc.vector.tensor_tensor(tn[:], tn[:], n2[:], op=Alu.subtract)
            n3 = aspool.tile([1, S], bf16, tag="n3")
            nc.scalar.copy(n3[:], tn[:])
            bs = bsp[coarse]
            b21 = abpool.tile([21, S], bf16, tag="b21")
            for r, j in enumerate([0, 1, 0, 2, 0, 1]):
                nc.sync.dma_start(b21[r * 3:(r + 1) * 3, :], bs[j][:, :])
            nc.sync.dma_start(b21[18:19, :], n1[:])
            nc.sync.dma_start(b21[19:20, :], n2[:])
            nc.sync.dma_start(b21[20:21, :], n3[:])

            a_s = asp[fine]
            a21 = abpool.tile([21, nf], bf16, tag="a21")
            for r, j in enumerate([0, 0, 1, 0, 2, 1]):
                nc.sync.dma_start(a21[r * 3:(r + 1) * 3, :], a_s[j][:, :])
            nc.sync.dma_start(a21[18:21, :], onesd[:, :nf])

            # query norms + eps, row layout: point t*128+p at [p, t]
            rx = smpool.tile([P, T * 3], f32, tag="rx")
            nc.sync.dma_start(
                rx[:].rearrange("p (t c) -> p t c", c=3),
                xr[fine][:, :].rearrange("(t p) c -> p t c", p=P))
            sqa = smpool.tile([P, T * 3], f32, tag="sqa")
            nc.vector.tensor_tensor(sqa[:], rx[:], rx[:], op=Alu.mult)
            nanrm = smpool.tile([P, T], f32, tag="nanrm")
            nc.vector.tensor_reduce(
                nanrm[:], sqa[:].rearrange("p (t c) -> p t c", c=3),
                axis=Axis.X, op=Alu.add)
            nc.vector.tensor_scalar_mul(nanrm[:], nanrm[:], -1.0)

            # per-stage arrays
            maxb = smpool.tile([P, T * 8], f32, tag="maxb")
            idxb = smpool.tile([P, T * 8], u32, tag="idxb")
            d3 = smpool.tile([P, T * KNN], f32, tag="d3")
            w3 = smpool.tile([P, T * KNN], f32, tag="w3")
            wgt = smpool.tile([P, T * KNN], f32, tag="wgt")
            wsum = smpool.tile([P, T], f32, tag="wsum")
            idxf = smpool.tile([P, T * KNN], f32, tag="idxf")
            return dict(a21=a21, b21=b21, nanrm=nanrm, maxb=maxb, idxb=idxb,
                        d3=d3, w3=w3, wgt=wgt, wsum=wsum, idxf=idxf)

        def emit_loop(s, cx_):
            st = stages[s]
            nf, S, D2 = st["nf"], st["S"], st["d2"]
            T = nf // P
            a21, b21, nanrm = cx_["a21"], cx_["b21"], cx_["nanrm"]
            maxb, idxb = cx_["maxb"], cx_["idxb"]
            d3, w3, wgt, wsum = cx_["d3"], cx_["w3"], cx_["wgt"], cx_["wsum"]
            idxf = cx_["idxf"]

            # software-pipelined tile loop: slot i runs the front half
            # (distances, top-3, gather launch, weights, diag) for tile i and
            # the back half (PE combine, psum copies, row write) for tile
            # i-LAG, so no engine's in-order stream sits behind work whose
            # inputs (the gathers) are still in flight.
            gts, diags = {}, {}
            for i in range(T + LAG):
                if i < T:
                    t = i
                    # nd_sb holds -d in fp16: the per-query |a|^2 is folded in
                    # as an ACT bias during the PSUM drain, so small distances
                    # keep full fp16 relative precision.
                    nd_sb = ndpool.tile([P, max(S, 512)], f16, tag="nd")
                    for h0 in range(0, S, 1024):
                        hw_ = min(1024, S - h0)
                        psd = ps_d.tile([P, 1024], f32, tag="psd")
                        for c0 in range(0, hw_, 512):
                            w = min(512, hw_ - c0)
                            nc.tensor.matmul(
                                psd[:, c0:c0 + w],
                                a21[:, t * P:(t + 1) * P],
                                b21[:, h0 + c0:h0 + c0 + w],
                                start=True, stop=True)
                        nc.scalar.activation(
                            nd_sb[:, h0:h0 + hw_], psd[:, :hw_],
                            mybir.ActivationFunctionType.Identity,
                            bias=nanrm[:, t:t + 1], scale=1.0)
                    nc.vector.max(maxb[:, t * 8:(t + 1) * 8], nd_sb[:, :S])
                    nc.vector.max_index(idxb[:, t * 8:(t + 1) * 8],
                                        maxb[:, t * 8:(t + 1) * 8],
                                        nd_sb[:, :S])
                    if s == 3:
                        gt = gpool.tile([P, KNN * D2], f16, tag="gt")
                        gts[t] = gt
                        for k in range(KNN):
                            nc.gpsimd.indirect_dma_start(
                                out=gt[:, k * D2:(k + 1) * D2],
                                out_offset=None,
                                in_=gsrc[s][:, :],
                                in_offset=IndirectOffsetOnAxis(
                                    ap=idxb[:, t * 8 + k:t * 8 + k + 1],
                                    axis=0))
                    # per-tile weights: maxb = -d, so d3 = -maxb + eps
                    t3 = slice(t * KNN, (t + 1) * KNN)
                    nc.vector.tensor_scalar(
                        d3[:, t3], maxb[:, t * 8:t * 8 + KNN], -1.0,
                        EPS, op0=Alu.mult, op1=Alu.add)
                    nc.vector.tensor_scalar_max(d3[:, t3], d3[:, t3], DFLOOR)
                    nc.vector.reciprocal(w3[:, t3], d3[:, t3])
                    nc.vector.tensor_reduce(
                        wsum[:, t:t + 1],
                        w3[:, t3].rearrange("p (o e) -> p o e", o=1),
                        axis=Axis.X, op=Alu.add)
                    nc.vector.reciprocal(wsum[:, t:t + 1], wsum[:, t:t + 1])
                    nc.vector.tensor_scalar_mul(wgt[:, t3], w3[:, t3],
                                                wsum[:, t:t + 1])
                    if s == 3:
                        diag = dpool.tile([P, KNN * P], f16, tag="diag")
                        diags[t] = diag
                        for k in range(KNN):
                            nc.vector.tensor_scalar_mul(
                                diag[:, k * P:(k + 1) * P], ident[:],
                                wgt[:, t * KNN + k:t * KNN + k + 1])
                    else:
                        # dense stages: build W[q, s] = sum_k w_k (s==idx_k)
                        # by iota-compare, then W^T via PE transpose.
                        nc.scalar.copy(idxf[:, t3], idxb[:, t * 8:t * 8 + 3])
                        W = wpool.tile([P, 512], f16, tag="W")
                        Wk = wpool.tile([P, 512], f16, tag="Wk")
                        nc.vector.tensor_scalar(
                            W[:, :S], iota16[:, :S],
                            idxf[:, t * KNN:t * KNN + 1],
                            wgt[:, t * KNN:t * KNN + 1],
                            op0=Alu.is_equal, op1=Alu.mult)
                        for k in (1, 2):
                            nc.vector.tensor_scalar(
                                Wk[:, :S], iota16[:, :S],
                                idxf[:, t * KNN + k:t * KNN + k + 1],
                                wgt[:, t * KNN + k:t * KNN + k + 1],
                                op0=Alu.is_equal, op1=Alu.mult)
                            nc.vector.tensor_tensor(W[:, :S], W[:, :S],
                                                    Wk[:, :S], op=Alu.add)
                        nblk = (S + P - 1) // P
                        wt = wtpool.tile([P, 512], f16, tag="wt")
                        diags[t] = wt
                        for blk in range(nblk):
                            bw = min(P, S - blk * P)
                            pst = ps_t.tile([P, 512], f16, tag="pst")
                            nc.tensor.transpose(
                                pst[:bw, :P], W[:, blk * P:blk * P + bw],
                                ident[:])
                            nc.scalar.copy(wt[:bw, blk * P:(blk + 1) * P],
                                           pst[:bw, :P])
                if i >= LAG:
                    t = i - LAG
                    if s == 3:
                        gt, diag = gts.pop(t), diags.pop(t)
                        res = rpool.tile([P, D2], f16, tag="res")
                        for h0 in range(0, D2, 1024):
                            hw_ = min(1024, D2 - h0)
                            pcb = ps_cb.tile([P, 1024], f32, tag="pcb")
                            for c0 in range(0, hw_, 512):
                                w = min(512, hw_ - c0)
                                for k in range(KNN):
                                    nc.tensor.matmul(
                                        pcb[:, c0:c0 + w],
                                        diag[:, k * P:(k + 1) * P],
                                        gt[:, k * D2 + h0 + c0:
                                           k * D2 + h0 + c0 + w],
                                        start=(k == 0), stop=(k == KNN - 1))
                            nc.scalar.copy(res[:, h0:h0 + hw_], pcb[:, :hw_])
                        nc.sync.dma_start(oi[t * P:(t + 1) * P, :], res[:])
                    else:
                        # dense interp: out[q, ch] = sum_s W^T[s, q] Tsb[s, ch]
                        wt = diags.pop(t)
                        nblk = (S + P - 1) // P
                        Dint = D2  # interp width produced by this stage
                        if s == 2:
                            res = rpool.tile([P, Dint], f16, tag="res")
                        for h0 in range(0, Dint, 1024):
                            hw_ = min(1024, Dint - h0)
                            pcb = ps_cb.tile([P, 1024], f32, tag="pcb")
                            for c0 in range(0, hw_, 512):
                                w = min(512, hw_ - c0)
                                for blk in range(nblk):
                                    bw = min(P, S - blk * P)
                                    if s == 0:
                                        rhs = t0sb[0:bw, h0 + c0:h0 + c0 + w]
                                    elif s == 1:
                                        rhs = t1sb[0:bw, h0 + c0:h0 + c0 + w]
                                    else:
                                        rhs = t2sb[0:bw,
                                                   blk * 1792 + h0 + c0:
                                                   blk * 1792 + h0 + c0 + w]
                                    nc.tensor.matmul(
                                        pcb[:, c0:c0 + w],
                                        wt[0:bw, blk * P:blk * P + P],
                                        rhs,
                                        start=(blk == 0),
                                        stop=(blk == nblk - 1))
                            # drain into the next stage's SBUF table (or res
                            # for stage 2, whose table lives in DRAM for the
                            # stage-3 gather)
                            if s == 0:
                                nc.scalar.copy(
                                    t1sb[:, 512 + h0:512 + h0 + hw_],
                                    pcb[:, :hw_])
                            elif s == 1:
                                nc.scalar.copy(
                                    t2sb[:, t * 1792 + 256 + h0:
                                         t * 1792 + 256 + h0 + hw_],
                                    pcb[:, :hw_])
                            else:
                                nc.scalar.copy(res[:, h0:h0 + hw_],
                                               pcb[:, :hw_])
                        if s == 2:
                            nc.sync.dma_start(
                                tsrc[3][t * P:(t + 1) * P,
                                        cs[1]:cs[1] + Dint], res[:])

        cx0 = emit_setup(0)
        cx1 = emit_setup(1)
        emit_loop(0, cx0)
        cx2 = emit_setup(2)
        emit_loop(1, cx1)
        cx3 = emit_setup(3)
        emit_loop(2, cx2)
        emit_loop(3, cx3)
    if split_waits:
        _split_multi_waits(nc)
    return nc


def _split_multi_waits(nc):
    """This walrus build rejects instructions carrying more than one sync
    wait. Hoist extra waits into same-engine NoOps inserted just before."""
    import concourse.mybir as mybir

    n = 0
    for f in nc.m.functions:
        for bb in f.blocks:
            il = bb.instructions
            i = 0
            while i < len(il):
                inst = il[i]
                si = getattr(inst, "sync_info", None)
                ow = list(si.on_wait) if si is not None else []
                if len(ow) > 1:
                    for w in ow[:-1]:
                        nop = mybir.InstNoOp(name=f"W{n}-{inst.name}",
                                             ins=[], outs=[])
                        n += 1
                        nop.engine = inst.engine
                        nop.sync_info = mybir.SyncInfo(on_update=[],
                                                       on_wait=[w])
                        il.insert(i, nop)
                        i += 1
                    inst.sync_info = mybir.SyncInfo(
                        on_update=list(si.on_update), on_wait=[ow[-1]])
                i += 1


def _patch_tile_drain():
    """This walrus build rejects >1 sync-wait on the kernel-tail Drain; spread
    the waits across single-wait SP nops instead."""
    import concourse.mybir as mybir
    import concourse.tile as tile
    from concourse.vector_clock import ScopedClock

    if getattr(tile.TileContext, "_drain_patched", False):
        return

    def _patched(self, tick_clock, wait_clock):
        nc = self.nc
        probe = nc.sync.nop()
        wait_clock.add_sem_waits(probe.ins,
                                 ScopedClock({None: tick_clock.global_clock}))
        si = probe.ins.sync_info
        ow = list(si.on_wait) if si is not None else []
        if len(ow) > 1:
            for w in ow[1:]:
                n2 = nc.sync.nop()
                n2.ins.sync_info = mybir.SyncInfo(on_update=[], on_wait=[w])
            probe.ins.sync_info = mybir.SyncInfo(on_update=list(si.on_update),
                                                 on_wait=[ow[0]])
        nc.sync.drain()
        nc.all_engine_barrier()
        assert self.sems is not None
        popped = nc._tile_sem_poison_stack.pop()
        assert popped is self._sem_poison
        nc.clear_and_free_semaphores(list(self.sems.allocated().values()))
        nc.all_engine_barrier()

    tile.TileContext._drain_and_barrier = _patched
    tile.TileContext._drain_patched = True


def _get_program(ns, cs, n_half):
    key = (tuple(ns), tuple(cs), n_half)
    if _CACHED["key"] != key:
        _CACHED["nc"] = _build_program(ns, cs, n_half)
        _CACHED["key"] = key
    return _CACHED["nc"]


def _split3(x):
    """Three-way bf16 split of an f32 array: x ~ s0+s1+s2 to ~2^-27."""
    import ml_dtypes
    s0 = x.astype(ml_dtypes.bfloat16)
    r = x - s0.astype(np.float32)
    s1 = r.astype(ml_dtypes.bfloat16)
    r = r - s1.astype(np.float32)
    s2 = r.astype(ml_dtypes.bfloat16)
    return [np.ascontiguousarray(s) for s in (s0, s1, s2)]


def make_core_inputs(inputs, ns, n_half, core):
    """Slice/transform full inputs for one core (b = core//2, half = core%2)."""
    import ml_dtypes
    b, h = core // 2, core % 2
    d = {}
    x0h = np.ascontiguousarray(
        np.asarray(inputs["xyz0"])[b, h * n_half:(h + 1) * n_half])
    d["xt0"] = np.ascontiguousarray(x0h.T)
    d["xr0"] = x0h
    for j, s in enumerate(_split3(2.0 * d["xt0"])):
        d[f"a{j}_0"] = s
    for i in range(1, 5):
        xi = np.ascontiguousarray(np.asarray(inputs[f"xyz{i}"])[b])
        d[f"xt{i}"] = np.ascontiguousarray(xi.T)
        d[f"xr{i}"] = xi
        for j, s in enumerate(_split3(2.0 * d[f"xt{i}"])):
            d[f"a{j}_{i}"] = s
        for j, s in enumerate(_split3(d[f"xt{i}"])):
            d[f"b{j}_{i}"] = s
        d[f"ft{i}"] = np.ascontiguousarray(
            np.asarray(inputs[f"x{i}"])[b].T.astype(np.float16))
    d["ones"] = np.ones((3, max(n_half, ns[1])), ml_dtypes.bfloat16)
    return d


def kernel(**inputs):
    from concourse.bass_utils import run_bass_kernel_spmd

    ns, cs = NS, CS
    n_half = ns[0] // 2
    nc = _get_program(ns, cs, n_half)

    in_maps = [make_core_inputs(inputs, ns, n_half, c) for c in range(8)]
    res = run_bass_kernel_spmd(nc, in_maps, core_ids=list(range(8)))

    dout = sum(cs)
    out = np.empty((B, dout, ns[0]), np.float32)
    out[:, :cs[0], :] = np.asarray(inputs["x0"])
    for c in range(8):
        b, h = c // 2, c % 2
        out[b, cs[0]:, h * n_half:(h + 1) * n_half] = \
            res.results[c]["oi"].astype(np.float32).T
    return out
